# revision 4
# baseline (speedup 1.0000x reference)
"""Multi-head attention (B=2, S=2048, D=2048, H=16, hd=128) on 8 TRN2 NeuronCores.

Sharding: data-parallel over batch (2) x tensor-parallel over head groups (4).
Core c handles batch c//4 and heads [4*(c%4), 4*(c%4)+4). Each core computes
q/k/v projections for its 512 features, RoPE, full attention over S for its 4
heads, and a partial output projection y_partial = attn_local @ wo[:, cols].T.
Host sums the 4 partials per batch (no on-chip collectives).

All matmuls run in bf16 with fp32 PSUM accumulation. RoPE pairs are split
even/odd across the partition dim by permuting wq/wk rows host-side, so RoPE
is plain elementwise DVE work against [64, S] cos/sin tables. Scores are
computed transposed ([k, q]) so softmax(exp)@V needs no on-chip transposes;
the softmax denominator is accumulated on DVE and divided out after PV.
"""

import numpy as np
import ml_dtypes

B = 2
S = 2048
D = 2048
H = 16
HD = 128
P = 128
N_CORES = 8
H_LOC = 4          # heads per core
F = H_LOC * HD     # local features = 512
NCH = 4            # n-chunks of 512 over S
CH = S // NCH      # 512
DCH = D // P       # 16 contraction chunks
NT = S // P        # 16 row tiles

_BF16 = ml_dtypes.bfloat16


def _build_program():
    import concourse.bass as bass
    import concourse.mybir as mybir
    import concourse.tile as tile
    from concourse import bacc

    dt = mybir.dt
    nc = bacc.Bacc("TRN2", target_bir_lowering=False, debug=False,
                   num_devices=N_CORES)

    xT = nc.dram_tensor("xT", [D, S], dt.bfloat16, kind="ExternalInput").ap()
    wqT = nc.dram_tensor("wqT", [D, F], dt.bfloat16, kind="ExternalInput").ap()
    wkT = nc.dram_tensor("wkT", [D, F], dt.bfloat16, kind="ExternalInput").ap()
    wvT = nc.dram_tensor("wvT", [D, F], dt.bfloat16, kind="ExternalInput").ap()
    woT = nc.dram_tensor("woT", [F, D], dt.bfloat16, kind="ExternalInput").ap()
    ropeq = nc.dram_tensor("ropeq", [P, S], dt.float32, kind="ExternalInput").ap()
    ropek = nc.dram_tensor("ropek", [P, S], dt.float32, kind="ExternalInput").ap()
    y = nc.dram_tensor("y", [S, D], dt.float32, kind="ExternalOutput").ap()

    xT3 = xT.rearrange("(o p) n -> p o n", p=P)      # [128, 16, 2048]
    wqT3 = wqT.rearrange("(o p) f -> p o f", p=P)    # [128, 16, 512]
    wkT3 = wkT.rearrange("(o p) f -> p o f", p=P)
    wvT3 = wvT.rearrange("(o p) f -> p o f", p=P)
    woT3 = woT.rearrange("(o p) n -> p o n", p=P)    # [128, 4, 2048]
    y3 = y.rearrange("(o p) n -> p o n", p=P)        # [128, 16, 2048]

    with tile.TileContext(nc) as tc:
        with (
            tc.tile_pool(name="persist", bufs=1) as pp,
            tc.tile_pool(name="work", bufs=2) as wk,
        ):
            # Long-lived SBUF tensors
            qTp = pp.tile([P, H_LOC, S], dt.bfloat16, tag="qTp")
            kTp = pp.tile([P, H_LOC, S], dt.bfloat16, tag="kTp")
            v_sb = pp.tile([P, NT, F], dt.bfloat16, tag="v")
            attnT = pp.tile([P, H_LOC, S], dt.bfloat16, tag="attnT")
            wo_sb = pp.tile([P, H_LOC, D], dt.bfloat16, tag="wo")
            rq_sb = pp.tile([P, S], dt.float32, tag="ropeq")
            rk_sb = pp.tile([P, S], dt.float32, tag="ropek")
            ones = pp.tile([P, P], dt.float32, tag="ones")

            nc.sync.dma_start(wo_sb[:], woT3[:])
            nc.sync.dma_start(rq_sb[:], ropeq[:])
            nc.sync.dma_start(rk_sb[:], ropek[:])
            nc.any.memset(ones[:], 1.0)

            # ---- Phase 1: q/k/v projections + RoPE -----------------------
            with (
                tc.tile_pool(name="p1", bufs=1) as p1,
                tc.tile_pool(name="p1x", bufs=2) as p1x,
                tc.tile_pool(name="p1t", bufs=3) as p1t,
                tc.tile_pool(name="ps1", bufs=4, space="PSUM") as ps1,
            ):
                wq_sb = p1.tile([P, DCH, F], dt.bfloat16, tag="wq")
                wk_sb = p1.tile([P, DCH, F], dt.bfloat16, tag="wk")
                wv_sb = p1.tile([P, DCH, F], dt.bfloat16, tag="wv")
                nc.sync.dma_start(wq_sb[:], wqT3[:])
                nc.sync.dma_start(wk_sb[:], wkT3[:])
                nc.sync.dma_start(wv_sb[:], wvT3[:])

                for nchunk in range(NCH):
                    nsl = slice(nchunk * CH, (nchunk + 1) * CH)
                    xc = p1x.tile([P, DCH, CH], dt.bfloat16, tag="xc")
                    nc.sync.dma_start(xc[:], xT3[:, :, nsl])

                    # qT / kT feature tiles (one head = one 128-feature tile)
                    for w_sb, rp_sb, outT in ((wq_sb, rq_sb, qTp),
                                              (wk_sb, rk_sb, kTp)):
                        for h in range(H_LOC):
                            ps = ps1.tile([P, CH], dt.float32, tag="ps1")
                            for dc in range(DCH):
                                nc.tensor.matmul(
                                    ps[:],
                                    w_sb[:, dc, h * HD:(h + 1) * HD],
                                    xc[:, dc, :],
                                    start=(dc == 0), stop=(dc == DCH - 1),
                                )
                            # RoPE: partitions 0:64 = even pairs, 64:128 = odd
                            t1 = p1t.tile([P, CH], dt.float32, tag="t1")
                            t2 = p1t.tile([P, CH], dt.float32, tag="t2")
                            c_ap = rp_sb[0:64, nsl]
                            s_ap = rp_sb[64:128, nsl]
                            o_sl = outT[:, h, nsl]
                            # t1 = [qe*c ; qe*s], t2 = [qo*s ; qo*c]
                            nc.vector.tensor_mul(out=t1[0:64, :], in0=ps[0:64, :], in1=c_ap)
                            nc.vector.tensor_mul(out=t1[64:128, :], in0=ps[0:64, :], in1=s_ap)
                            nc.vector.tensor_mul(out=t2[0:64, :], in0=ps[64:128, :], in1=s_ap)
                            nc.vector.tensor_mul(out=t2[64:128, :], in0=ps[64:128, :], in1=c_ap)
                            nc.vector.tensor_sub(out=o_sl[0:64, :], in0=t1[0:64, :], in1=t2[0:64, :])
                            nc.vector.tensor_add(out=o_sl[64:128, :], in0=t1[64:128, :], in1=t2[64:128, :])

                    # v tiles (natural [n, f] layout)
                    for nt in range(NCH):
                        ps = ps1.tile([P, F], dt.float32, tag="ps1")
                        for dc in range(DCH):
                            nc.tensor.matmul(
                                ps[:],
                                xc[:, dc, nt * P:(nt + 1) * P],
                                wv_sb[:, dc, :],
                                start=(dc == 0), stop=(dc == DCH - 1),
                            )
                        nc.scalar.activation(
                            v_sb[:, nchunk * NCH + nt, :], ps[:],
                            mybir.ActivationFunctionType.Copy)

            # ---- Phase 2: attention -------------------------------------
            with (
                tc.tile_pool(name="p2e", bufs=4) as p2e,
                tc.tile_pool(name="p2a", bufs=2) as p2a,
                tc.tile_pool(name="p2r", bufs=2) as p2r,
                tc.tile_pool(name="ps_s", bufs=3, space="PSUM") as ps_s,
                tc.tile_pool(name="ps_pv", bufs=2, space="PSUM") as ps_pv,
                tc.tile_pool(name="ps_b", bufs=1, space="PSUM") as ps_b,
            ):
                for qc in range(NCH):
                    qsl = slice(qc * CH, (qc + 1) * CH)
                    for h in range(H_LOC):
                        pv = ps_pv.tile([P, CH], dt.float32, tag="pv")
                        acc = p2a.tile([P, CH], dt.float32, tag="acc")
                        for kt in range(NT):
                            ss = ps_s.tile([P, CH], dt.float32, tag="ss")
                            nc.tensor.matmul(
                                ss[:],
                                kTp[:, h, kt * P:(kt + 1) * P],
                                qTp[:, h, qsl],
                                start=True, stop=True,
                            )
                            et = p2e.tile([P, CH], dt.bfloat16, tag="et")
                            nc.scalar.activation(
                                et[:], ss[:], mybir.ActivationFunctionType.Exp)
                            nc.tensor.matmul(
                                pv[:],
                                v_sb[:, kt, h * HD:(h + 1) * HD],
                                et[:],
                                start=(kt == 0), stop=(kt == NT - 1),
                            )
                            if kt == 0:
                                nc.vector.tensor_copy(acc[:], et[:])
                            else:
                                nc.vector.tensor_add(out=acc[:], in0=acc[:], in1=et[:])
                        # fold 128 partitions -> 1 via fp32 ones-matmul on PE
                        pd = ps_b.tile([1, CH], dt.float32, tag="pd")
                        nc.tensor.matmul(pd[:], ones[:, 0:1], acc[:],
                                         start=True, stop=True)
                        rec = p2r.tile([1, CH], dt.float32, tag="rec")
                        nc.vector.reciprocal(rec[:], pd[:])
                        bb = ps_b.tile([P, CH], dt.float32, tag="bb")
                        nc.tensor.matmul(bb[:], ones[0:1, :], rec[:],
                                         start=True, stop=True)
                        bc = p2a.tile([P, CH], dt.float32, tag="bc")
                        nc.scalar.activation(
                            bc[:], bb[:], mybir.ActivationFunctionType.Copy)
                        nc.vector.tensor_mul(
                            out=attnT[:, h, qsl], in0=pv[:], in1=bc[:])

            # ---- Phase 3: output projection ------------------------------
            with (
                tc.tile_pool(name="p3", bufs=4) as p3,
                tc.tile_pool(name="ps_y", bufs=4, space="PSUM") as ps_y,
            ):
                for nt in range(NT):
                    for oc in range(NCH):
                        osl = slice(oc * CH, (oc + 1) * CH)
                        py = ps_y.tile([P, CH], dt.float32, tag="py")
                        for h in range(H_LOC):
                            nc.tensor.matmul(
                                py[:],
                                attnT[:, h, nt * P:(nt + 1) * P],
                                wo_sb[:, h, osl],
                                start=(h == 0), stop=(h == H_LOC - 1),
                            )
                        yt = p3.tile([P, CH], dt.float32, tag="yt")
                        nc.scalar.activation(
                            yt[:], py[:], mybir.ActivationFunctionType.Copy)
                        nc.sync.dma_start(y3[:, nt, osl], yt[:])

    nc.compile()
    return nc


_NC_CACHE = None


def _get_program():
    global _NC_CACHE
    if _NC_CACHE is None:
        _NC_CACHE = _build_program()
    return _NC_CACHE


def _rope_tables():
    scale = np.arange(0, HD, 2, dtype=np.float32) / HD
    inv_freq = 1.0 / (10000.0 ** scale)                 # [64]
    t = np.arange(S, dtype=np.float32)
    ang = np.outer(t, inv_freq)                         # [S, 64]
    cos = np.cos(ang).T.astype(np.float32)              # [64, S]
    sin = np.sin(ang).T.astype(np.float32)
    sc = np.float32(1.0 / np.sqrt(HD))
    ropeq = np.concatenate([cos * sc, sin * sc], axis=0)  # [128, S]
    ropek = np.concatenate([cos, sin], axis=0)
    return np.ascontiguousarray(ropeq), np.ascontiguousarray(ropek)


def prepare_in_maps(x, wq, wk, wv, wo):
    x = np.asarray(x, dtype=np.float32)
    wq = np.asarray(wq, dtype=np.float32)
    wk = np.asarray(wk, dtype=np.float32)
    wv = np.asarray(wv, dtype=np.float32)
    wo = np.asarray(wo, dtype=np.float32)

    ropeq, ropek = _rope_tables()

    # even/odd RoPE permutation of rows within each head
    perm = np.concatenate([np.arange(0, HD, 2), np.arange(1, HD, 2)])

    xT = [np.ascontiguousarray(x[b].T).astype(_BF16) for b in range(B)]

    in_maps = []
    for c in range(N_CORES):
        b, hg = divmod(c, H_LOC)
        heads = np.arange(hg * H_LOC, (hg + 1) * H_LOC)
        rows_qk = (heads[:, None] * HD + perm[None, :]).reshape(-1)  # [512]
        rows_nat = np.arange(hg * F, (hg + 1) * F)
        in_maps.append({
            "xT": xT[b],
            "wqT": np.ascontiguousarray(wq[rows_qk].T).astype(_BF16),
            "wkT": np.ascontiguousarray(wk[rows_qk].T).astype(_BF16),
            "wvT": np.ascontiguousarray(wv[rows_nat].T).astype(_BF16),
            "woT": np.ascontiguousarray(wo[:, rows_nat].T).astype(_BF16),
            "ropeq": ropeq,
            "ropek": ropek,
        })
    return in_maps


def combine_results(results):
    out = np.zeros((B, S, D), dtype=np.float32)
    for c, r in enumerate(results):
        out[c // H_LOC] += r["y"]
    return out


def kernel(x, wq, wk, wv, wo):
    from concourse.bass_utils import run_bass_kernel_spmd

    nc = _get_program()
    in_maps = prepare_in_maps(x, wq, wk, wv, wo)
    res = run_bass_kernel_spmd(nc, in_maps, core_ids=list(range(N_CORES)))
    return combine_results(res.results)


if __name__ == "__main__":
    rng = np.random.default_rng(0)
    ins = {
        "x": rng.standard_normal((B, S, D), dtype=np.float32),
        "wq": rng.standard_normal((D, D), dtype=np.float32) / np.sqrt(D),
        "wk": rng.standard_normal((D, D), dtype=np.float32) / np.sqrt(D),
        "wv": rng.standard_normal((D, D), dtype=np.float32) / np.sqrt(D),
        "wo": rng.standard_normal((D, D), dtype=np.float32) / np.sqrt(D),
    }
    out = kernel(**ins)
    print("out", out.shape, out.dtype, np.abs(out).max())


# revision 6
# speedup vs baseline: 1.0882x; 1.0882x over previous
"""Multi-head attention (B=2, S=2048, D=2048, H=16, hd=128) on 8 TRN2 NeuronCores.

Sharding: data-parallel over batch (2) x tensor-parallel over head groups (4).
Core c handles batch c//4 and heads [4*(c%4), 4*(c%4)+4). Each core computes
q/k/v projections for its 512 features, RoPE, full attention over S for its 4
heads, and a partial output projection y_partial = attn_local @ wo[:, cols].T.
Host sums the 4 partials per batch (no on-chip collectives).

All matmuls run in f16 with fp32 PSUM accumulation. RoPE pairs are split
even/odd across the partition dim by permuting wq/wk rows host-side, so RoPE
is elementwise DVE work against stacked [cos;cos] / [sin;sin] tables. Scores
are computed transposed ([k, q]) so softmax(exp)@V needs no on-chip
transposes; the softmax denominator is accumulated on DVE, all-reduced across
partitions on GpSimd, and divided out after the PV matmul.

Phase order: k (all chunks) -> v -> per-q-chunk (q proj + attention) ->
output projection, so ScalarE exp work overlaps TensorE matmul work.
"""

import numpy as np
import ml_dtypes

B = 2
S = 2048
D = 2048
H = 16
HD = 128
P = 128
N_CORES = 8
H_LOC = 4          # heads per core
F = H_LOC * HD     # local features = 512
NCH = 4            # n-chunks of 512 over S
CH = S // NCH      # 512
DCH = D // P       # 16 contraction chunks
NT = S // P        # 16 row tiles

_F16 = np.float16


def _build_program():
    import concourse.bass as bass
    import concourse.bass_isa as bass_isa
    import concourse.mybir as mybir
    import concourse.tile as tile
    from concourse import bacc

    dt = mybir.dt
    nc = bacc.Bacc("TRN2", target_bir_lowering=False, debug=False,
                   num_devices=N_CORES)

    xT = nc.dram_tensor("xT", [D, S], dt.float16, kind="ExternalInput").ap()
    wqT = nc.dram_tensor("wqT", [D, F], dt.float16, kind="ExternalInput").ap()
    wkT = nc.dram_tensor("wkT", [D, F], dt.float16, kind="ExternalInput").ap()
    wvT = nc.dram_tensor("wvT", [D, F], dt.float16, kind="ExternalInput").ap()
    woT = nc.dram_tensor("woT", [F, D], dt.float16, kind="ExternalInput").ap()
    # stacked RoPE tables: [cos;cos] and [sin;sin], q-variant scaled by 1/sqrt(hd)
    cq = nc.dram_tensor("cq", [P, S], dt.float16, kind="ExternalInput").ap()
    sq = nc.dram_tensor("sq", [P, S], dt.float16, kind="ExternalInput").ap()
    ck = nc.dram_tensor("ck", [P, S], dt.float16, kind="ExternalInput").ap()
    sk = nc.dram_tensor("sk", [P, S], dt.float16, kind="ExternalInput").ap()
    y = nc.dram_tensor("y", [S, D], dt.float32, kind="ExternalOutput").ap()

    xT3 = xT.rearrange("(o p) n -> p o n", p=P)      # [128, 16, 2048]
    wqT3 = wqT.rearrange("(o p) f -> p o f", p=P)    # [128, 16, 512]
    wkT3 = wkT.rearrange("(o p) f -> p o f", p=P)
    wvT3 = wvT.rearrange("(o p) f -> p o f", p=P)
    woT3 = woT.rearrange("(o p) n -> p o n", p=P)    # [128, 4, 2048]
    y3 = y.rearrange("(o p) n -> p o n", p=P)        # [128, 16, 2048]

    with tile.TileContext(nc) as tc:
        with tc.tile_pool(name="persist", bufs=1) as pp:
            # Long-lived SBUF tensors
            qTp = pp.tile([P, H_LOC, S], dt.float16, tag="qTp")
            kTp = pp.tile([P, H_LOC, S], dt.float16, tag="kTp")
            v_sb = pp.tile([P, NT, F], dt.float16, tag="v")
            attnT = pp.tile([P, H_LOC, S], dt.float16, tag="attnT")
            wk_sb = pp.tile([P, DCH, F], dt.float16, tag="wk")
            wv_sb = pp.tile([P, DCH, F], dt.float16, tag="wv")
            wq_sb = pp.tile([P, DCH, F], dt.float16, tag="wq")
            wo_sb = pp.tile([P, H_LOC, D], dt.float16, tag="wo")
            cq_sb = pp.tile([P, S], dt.float16, tag="cq")
            sq_sb = pp.tile([P, S], dt.float16, tag="sq")
            ck_sb = pp.tile([P, S], dt.float16, tag="ck")
            sk_sb = pp.tile([P, S], dt.float16, tag="sk")

            # DMA issue order = need order: wk first (phase A), x chunks come
            # inside the phase loops, then tables, wv (B), wq (C), wo (D).
            nc.sync.dma_start(wk_sb[:], wkT3[:])
            nc.sync.dma_start(ck_sb[:], ck[:])
            nc.sync.dma_start(sk_sb[:], sk[:])
            nc.sync.dma_start(wv_sb[:], wvT3[:])
            nc.sync.dma_start(cq_sb[:], cq[:])
            nc.sync.dma_start(sq_sb[:], sq[:])
            nc.sync.dma_start(wq_sb[:], wqT3[:])
            nc.sync.dma_start(wo_sb[:], woT3[:])

            def rope(ps_pool, tmp_pool, ps, cs, sn, outT, h, nsl, t1_tag="t1"):
                # partitions 0:64 even pairs (e), 64:128 odd (o); pair i:
                #   out_e = e*c - o*s ; out_o = e*s + o*c
                # t1 = ps*[c;c] (PSUM), t2 = ps*[s;s] (SBUF) then cross-halves.
                t1 = ps_pool.tile([P, CH], dt.float32, tag=t1_tag)
                t2 = tmp_pool.tile([P, CH], dt.float16, tag="t2")
                nc.vector.tensor_mul(out=t1[:], in0=ps[:], in1=cs[:, nsl])
                nc.vector.tensor_mul(out=t2[:], in0=ps[:], in1=sn[:, nsl])
                o_sl = outT[:, h, nsl]
                nc.vector.tensor_sub(out=o_sl[0:64, :], in0=t1[0:64, :],
                                     in1=t2[64:128, :])
                nc.vector.tensor_add(out=o_sl[64:128, :], in0=t2[0:64, :],
                                     in1=t1[64:128, :])

            # ---- Phase A: kT projections + RoPE --------------------------
            with (
                tc.tile_pool(name="pax", bufs=2) as pax,
                tc.tile_pool(name="pat", bufs=3) as pat,
                tc.tile_pool(name="psa", bufs=1, space="PSUM") as psa,
            ):
                for nchunk in range(NCH):
                    nsl = slice(nchunk * CH, (nchunk + 1) * CH)
                    xc = pax.tile([P, DCH, CH], dt.float16, tag="xc")
                    nc.sync.dma_start(xc[:], xT3[:, :, nsl])
                    for h in range(H_LOC):
                        ps = psa.tile([P, CH], dt.float32, tag="ps", bufs=3)
                        for dc in range(DCH):
                            nc.tensor.matmul(
                                ps[:], wk_sb[:, dc, h * HD:(h + 1) * HD],
                                xc[:, dc, :],
                                start=(dc == 0), stop=(dc == DCH - 1))
                        rope(psa, pat, ps, ck_sb, sk_sb, kTp, h, nsl)

            # ---- Phase B: v projections (natural [n, f] layout) ----------
            with (
                tc.tile_pool(name="pbx", bufs=2) as pbx,
                tc.tile_pool(name="psb", bufs=3, space="PSUM") as psb,
            ):
                for nchunk in range(NCH):
                    nsl = slice(nchunk * CH, (nchunk + 1) * CH)
                    xc = pbx.tile([P, DCH, CH], dt.float16, tag="xc")
                    nc.sync.dma_start(xc[:], xT3[:, :, nsl])
                    for nt in range(NCH):
                        ps = psb.tile([P, CH], dt.float32, tag="ps")
                        for dc in range(DCH):
                            nc.tensor.matmul(
                                ps[:], xc[:, dc, nt * P:(nt + 1) * P],
                                wv_sb[:, dc, :],
                                start=(dc == 0), stop=(dc == DCH - 1))
                        nc.scalar.activation(
                            v_sb[:, nchunk * NCH + nt, :], ps[:],
                            mybir.ActivationFunctionType.Copy)

            # ---- Phase C: per q-chunk: q proj + RoPE, then attention -----
            with (
                tc.tile_pool(name="pcx", bufs=2) as pcx,
                tc.tile_pool(name="pct", bufs=3) as pct,
                tc.tile_pool(name="pce", bufs=4) as pce,
                tc.tile_pool(name="pca", bufs=2) as pca,
                tc.tile_pool(name="psm", bufs=2, space="PSUM") as psm,
                tc.tile_pool(name="pss", bufs=2, space="PSUM") as pss,
                tc.tile_pool(name="psv", bufs=2, space="PSUM") as psv,
            ):
                for qc in range(NCH):
                    qsl = slice(qc * CH, (qc + 1) * CH)
                    xc = pcx.tile([P, DCH, CH], dt.float16, tag="xc")
                    nc.sync.dma_start(xc[:], xT3[:, :, qsl])
                    for h in range(H_LOC):
                        ps = psm.tile([P, CH], dt.float32, tag="misc")
                        for dc in range(DCH):
                            nc.tensor.matmul(
                                ps[:], wq_sb[:, dc, h * HD:(h + 1) * HD],
                                xc[:, dc, :],
                                start=(dc == 0), stop=(dc == DCH - 1))
                        rope(psm, pct, ps, cq_sb, sq_sb, qTp, h, qsl, t1_tag="misc")

                    for h in range(H_LOC):
                        hsl = slice(h * HD, (h + 1) * HD)
                        pv = psv.tile([P, CH], dt.float32, tag="pv")
                        acc = pca.tile([P, CH], dt.float16, tag="acc")
                        for ktp in range(NT // 2):
                            ss = pss.tile([P, 2, CH], dt.float32, tag="ss")
                            for i in range(2):
                                kt = 2 * ktp + i
                                nc.tensor.matmul(
                                    ss[:, i, :],
                                    kTp[:, h, kt * P:(kt + 1) * P],
                                    qTp[:, h, qsl],
                                    start=True, stop=True)
                            et = pce.tile([P, 2, CH], dt.float16, tag="et")
                            nc.scalar.activation(
                                et[:], ss[:], mybir.ActivationFunctionType.Exp)
                            for i in range(2):
                                kt = 2 * ktp + i
                                nc.tensor.matmul(
                                    pv[:], v_sb[:, kt, hsl], et[:, i, :],
                                    start=(kt == 0), stop=(kt == NT - 1))
                                if kt == 0:
                                    nc.vector.tensor_copy(acc[:], et[:, 0, :])
                                else:
                                    nc.vector.tensor_add(
                                        out=acc[:], in0=acc[:], in1=et[:, i, :])
                        allr = pca.tile([P, CH], dt.float32, tag="allr")
                        nc.gpsimd.partition_all_reduce(
                            allr[:], acc[:], channels=P,
                            reduce_op=bass_isa.ReduceOp.add)
                        rec = pca.tile([P, CH], dt.float32, tag="rec")
                        nc.vector.reciprocal_approx_fast(rec[:], allr[:])
                        nc.vector.tensor_mul(
                            out=attnT[:, h, qsl], in0=pv[:], in1=rec[:])

            # ---- Phase D: output projection ------------------------------
            with (
                tc.tile_pool(name="pd", bufs=3) as pd,
                tc.tile_pool(name="psy", bufs=2, space="PSUM") as psy,
            ):
                for nt in range(NT):
                    py = psy.tile([P, NCH, CH], dt.float32, tag="py")
                    for h in range(H_LOC):
                        for oc in range(NCH):
                            nc.tensor.matmul(
                                py[:, oc, :],
                                attnT[:, h, nt * P:(nt + 1) * P],
                                wo_sb[:, h, oc * CH:(oc + 1) * CH],
                                start=(h == 0), stop=(h == H_LOC - 1))
                    yt = pd.tile([P, D], dt.float32, tag="yt")
                    nc.scalar.activation(
                        yt[:], py[:], mybir.ActivationFunctionType.Copy)
                    nc.sync.dma_start(y3[:, nt, :], yt[:])

    nc.compile()
    return nc


_NC_CACHE = None


def _get_program():
    global _NC_CACHE
    if _NC_CACHE is None:
        _NC_CACHE = _build_program()
    return _NC_CACHE


def _rope_tables():
    scale = np.arange(0, HD, 2, dtype=np.float32) / HD
    inv_freq = 1.0 / (10000.0 ** scale)                 # [64]
    t = np.arange(S, dtype=np.float32)
    ang = np.outer(t, inv_freq)                         # [S, 64]
    cos = np.cos(ang).T.astype(np.float32)              # [64, S]
    sin = np.sin(ang).T.astype(np.float32)
    sc = np.float32(1.0 / np.sqrt(HD))
    stk = lambda a: np.ascontiguousarray(
        np.concatenate([a, a], axis=0)).astype(_F16)    # [128, S]
    return stk(cos * sc), stk(sin * sc), stk(cos), stk(sin)


def prepare_in_maps(x, wq, wk, wv, wo):
    x = np.asarray(x, dtype=np.float32)
    wq = np.asarray(wq, dtype=np.float32)
    wk = np.asarray(wk, dtype=np.float32)
    wv = np.asarray(wv, dtype=np.float32)
    wo = np.asarray(wo, dtype=np.float32)

    cq_t, sq_t, ck_t, sk_t = _rope_tables()

    # even/odd RoPE permutation of rows within each head
    perm = np.concatenate([np.arange(0, HD, 2), np.arange(1, HD, 2)])

    xT = [np.ascontiguousarray(x[b].T).astype(_F16) for b in range(B)]

    in_maps = []
    for c in range(N_CORES):
        b, hg = divmod(c, H_LOC)
        heads = np.arange(hg * H_LOC, (hg + 1) * H_LOC)
        rows_qk = (heads[:, None] * HD + perm[None, :]).reshape(-1)  # [512]
        rows_nat = np.arange(hg * F, (hg + 1) * F)
        in_maps.append({
            "xT": xT[b],
            "wqT": np.ascontiguousarray(wq[rows_qk].T).astype(_F16),
            "wkT": np.ascontiguousarray(wk[rows_qk].T).astype(_F16),
            "wvT": np.ascontiguousarray(wv[rows_nat].T).astype(_F16),
            "woT": np.ascontiguousarray(wo[:, rows_nat].T).astype(_F16),
            "cq": cq_t, "sq": sq_t, "ck": ck_t, "sk": sk_t,
        })
    return in_maps


def combine_results(results):
    out = np.zeros((B, S, D), dtype=np.float32)
    for c, r in enumerate(results):
        out[c // H_LOC] += r["y"]
    return out


def kernel(x, wq, wk, wv, wo):
    from concourse.bass_utils import run_bass_kernel_spmd

    nc = _get_program()
    in_maps = prepare_in_maps(x, wq, wk, wv, wo)
    res = run_bass_kernel_spmd(nc, in_maps, core_ids=list(range(N_CORES)))
    return combine_results(res.results)


if __name__ == "__main__":
    rng = np.random.default_rng(0)
    ins = {
        "x": rng.standard_normal((B, S, D), dtype=np.float32),
        "wq": rng.standard_normal((D, D), dtype=np.float32) / np.sqrt(D),
        "wk": rng.standard_normal((D, D), dtype=np.float32) / np.sqrt(D),
        "wv": rng.standard_normal((D, D), dtype=np.float32) / np.sqrt(D),
        "wo": rng.standard_normal((D, D), dtype=np.float32) / np.sqrt(D),
    }
    out = kernel(**ins)
    print("out", out.shape, out.dtype, np.abs(out).max())


# revision 9
# speedup vs baseline: 1.1915x; 1.0949x over previous
"""Multi-head attention (B=2, S=2048, D=2048, H=16, hd=128) on 8 TRN2 NeuronCores.

Sharding: data-parallel over batch (2) x tensor-parallel over head groups (4).
Core c handles batch c//4 and heads [4*(c%4), 4*(c%4)+4). Each core computes
q/k/v projections for its 512 features, RoPE, full attention over S for its 4
heads, and a partial output projection y_partial = attn_local @ wo[:, cols].T.
Host sums the 4 partials per batch (no on-chip collectives).

All matmuls run in f16 with fp32 PSUM accumulation. The 1/sqrt(hd) score
scale is folded into wq host-side. RoPE pairs are split even/odd across the
partition dim by permuting wq/wk rows host-side, so RoPE is elementwise DVE
work against stacked [cos;cos] / [sin;sin] tables. Scores are computed
transposed ([k, q]) so softmax(exp)@V needs no on-chip transposes; the
softmax denominator is accumulated on DVE, all-reduced across partitions on
GpSimd, and divided out after the PV matmul.

Emission order is a software pipeline that keeps TensorE dense: k proj,
q proj with the first two attention score blocks interleaved, v proj (exp
hides under the v GEMM), then steady-state
[pv(b) | scores(b+2) | projection(finished q-chunk)].
"""

import numpy as np

B = 2
S = 2048
D = 2048
H = 16
HD = 128
P = 128
N_CORES = 8
H_LOC = 4          # heads per core
F = H_LOC * HD     # local features = 512
NCH = 4            # n-chunks of 512 over S
CH = S // NCH      # 512
DCH = D // P       # 16 contraction chunks
NT = S // P        # 16 row tiles

_F16 = np.float16


def _build_program():
    import concourse.bass_isa as bass_isa
    import concourse.mybir as mybir
    import concourse.tile as tile
    from concourse import bacc

    dt = mybir.dt
    nc = bacc.Bacc("TRN2", target_bir_lowering=False, debug=False,
                   num_devices=N_CORES)

    xT = nc.dram_tensor("xT", [D, S], dt.float16, kind="ExternalInput").ap()
    wqT = nc.dram_tensor("wqT", [D, F], dt.float16, kind="ExternalInput").ap()
    wkT = nc.dram_tensor("wkT", [D, F], dt.float16, kind="ExternalInput").ap()
    wvT = nc.dram_tensor("wvT", [D, F], dt.float16, kind="ExternalInput").ap()
    woT = nc.dram_tensor("woT", [F, D], dt.float16, kind="ExternalInput").ap()
    # stacked RoPE tables: [cos;cos] and [sin;sin]
    ct = nc.dram_tensor("ct", [P, S], dt.float16, kind="ExternalInput").ap()
    st = nc.dram_tensor("st", [P, S], dt.float16, kind="ExternalInput").ap()
    y = nc.dram_tensor("y", [S, D], dt.float32, kind="ExternalOutput").ap()

    xT3 = xT.rearrange("(o p) n -> p o n", p=P)      # [128, 16, 2048]
    wqT3 = wqT.rearrange("(o p) f -> p o f", p=P)    # [128, 16, 512]
    wkT3 = wkT.rearrange("(o p) f -> p o f", p=P)
    wvT3 = wvT.rearrange("(o p) f -> p o f", p=P)
    woT3 = woT.rearrange("(o p) n -> p o n", p=P)    # [128, 4, 2048]
    y3 = y.rearrange("(o p) n -> p o n", p=P)        # [128, 16, 2048]

    NB = NCH * H_LOC  # 16 attention blocks, b = qc*4 + h

    with tile.TileContext(nc) as tc:
        with (
            tc.tile_pool(name="persist", bufs=1) as pp,
            tc.tile_pool(name="xcp", bufs=2) as xcp,
        ):
            qTp = pp.tile([P, H_LOC, S], dt.float16, tag="qTp")
            kTp = pp.tile([P, H_LOC, S], dt.float16, tag="kTp")
            v_sb = pp.tile([P, NT, F], dt.float16, tag="v")
            wv_sb = pp.tile([P, DCH, F], dt.float16, tag="wv")
            wo_sb = pp.tile([P, H_LOC, D], dt.float16, tag="wo")

            # ---- phase 1: k and q projections + RoPE ---------------------
            with (
                tc.tile_pool(name="wp", bufs=1) as wp,
                tc.tile_pool(name="t2p", bufs=3) as t2p,
                tc.tile_pool(name="psg", bufs=1, space="PSUM") as psg,
            ):
                wk_sb = wp.tile([P, DCH, F], dt.float16, tag="wk")
                wq_sb = wp.tile([P, DCH, F], dt.float16, tag="wq")
                ct_sb = wp.tile([P, S], dt.float16, tag="ct")
                st_sb = wp.tile([P, S], dt.float16, tag="st")

                # DMA issue order = need order.
                nc.sync.dma_start(wk_sb[:], wkT3[:])
                nc.sync.dma_start(ct_sb[:], ct[:])
                nc.sync.dma_start(st_sb[:], st[:])
                nc.sync.dma_start(wq_sb[:], wqT3[:])
                nc.sync.dma_start(wv_sb[:], wvT3[:])
                nc.sync.dma_start(wo_sb[:], woT3[:])

                def proj_rope(w_sb, outT, nchunk):
                    """One n-chunk of a q/k projection + RoPE into outT."""
                    nsl = slice(nchunk * CH, (nchunk + 1) * CH)
                    xc = xcp.tile([P, DCH, CH], dt.float16, tag="xc")
                    nc.sync.dma_start(xc[:], xT3[:, :, nsl])
                    for h in range(H_LOC):
                        ps = psg.tile([P, CH], dt.float32, tag="gemm", bufs=3)
                        for dc in range(DCH):
                            nc.tensor.matmul(
                                ps[:], w_sb[:, dc, h * HD:(h + 1) * HD],
                                xc[:, dc, :],
                                start=(dc == 0), stop=(dc == DCH - 1))
                        # RoPE: partitions 0:64 = even pairs e, 64:128 odd o:
                        #   out_e = e*c - o*s ; out_o = e*s + o*c
                        t1 = psg.tile([P, CH], dt.float32, tag="t1", bufs=2)
                        t2 = t2p.tile([P, CH], dt.float16, tag="t2")
                        nc.vector.tensor_mul(out=t1[:], in0=ps[:],
                                             in1=ct_sb[:, nsl])
                        nc.vector.tensor_mul(out=t2[:], in0=ps[:],
                                             in1=st_sb[:, nsl])
                        o_sl = outT[:, h, nsl]
                        nc.vector.tensor_sub(out=o_sl[0:64, :], in0=t1[0:64, :],
                                             in1=t2[64:128, :])
                        nc.vector.tensor_add(out=o_sl[64:128, :],
                                             in0=t2[0:64, :],
                                             in1=t1[64:128, :])

                for nchunk in range(NCH):
                    proj_rope(wk_sb, kTp, nchunk)
                for nchunk in range(NCH):
                    proj_rope(wq_sb, qTp, nchunk)

            # ---- phase 2: scores pipeline + v + pv + projection ----------
            with (
                tc.tile_pool(name="etp", bufs=16) as etp,
                tc.tile_pool(name="attnp", bufs=2) as attnp,
                tc.tile_pool(name="accp", bufs=3) as accp,
                tc.tile_pool(name="ytp", bufs=4) as ytp,
                tc.tile_pool(name="psc", bufs=1, space="PSUM") as psc,
            ):
                acc_of = {}

                def scores_block(b):
                    qc, h = divmod(b, H_LOC)
                    qsl = slice(qc * CH, (qc + 1) * CH)
                    ets = []
                    acc = accp.tile([P, 2, CH], dt.float16, tag="acc")
                    for ktp in range(NT // 2):
                        ss = psc.tile([P, 2, CH], dt.float32, tag="ss", bufs=2)
                        for i in range(2):
                            kt = 2 * ktp + i
                            nc.tensor.matmul(
                                ss[:, i, :], kTp[:, h, kt * P:(kt + 1) * P],
                                qTp[:, h, qsl], start=True, stop=True)
                        et = etp.tile([P, 2, CH], dt.float16, tag="et")
                        nc.scalar.activation(
                            et[:], ss[:], mybir.ActivationFunctionType.Exp)
                        if ktp == 0:
                            nc.vector.tensor_copy(acc[:], et[:])
                        else:
                            nc.vector.tensor_add(out=acc[:], in0=acc[:],
                                                 in1=et[:])
                        ets.append(et)
                    acc_of[b] = (acc, ets)

                def pv_block(b, attn_cur):
                    qc, h = divmod(b, H_LOC)
                    hsl = slice(h * HD, (h + 1) * HD)
                    acc, ets = acc_of.pop(b)
                    pv = psc.tile([P, CH], dt.float32, tag="pv", bufs=2)
                    for ktp in range(NT // 2):
                        et = ets[ktp]
                        for i in range(2):
                            kt = 2 * ktp + i
                            nc.tensor.matmul(
                                pv[:], v_sb[:, kt, hsl], et[:, i, :],
                                start=(kt == 0), stop=(kt == NT - 1))
                    denom = accp.tile([P, CH], dt.float32, tag="den", bufs=2)
                    nc.vector.tensor_add(out=denom[:], in0=acc[:, 0, :],
                                         in1=acc[:, 1, :])
                    allr = accp.tile([P, CH], dt.float32, tag="allr", bufs=2)
                    nc.gpsimd.partition_all_reduce(
                        allr[:], denom[:], channels=P,
                        reduce_op=bass_isa.ReduceOp.add)
                    rec = accp.tile([P, CH], dt.float32, tag="rec", bufs=2)
                    nc.vector.reciprocal_approx_fast(rec[:], allr[:])
                    nc.vector.tensor_mul(
                        out=attn_cur[:, h, :], in0=pv[:], in1=rec[:])

                def proj_chunk(qc, attn_cur):
                    for ntl in range(NCH):
                        nt = qc * NCH + ntl
                        for half in range(2):
                            yt = ytp.tile([P, D // 2], dt.float32, tag="yt")
                            for i in range(2):
                                oc = half * 2 + i
                                py = psc.tile([P, CH], dt.float32, tag="py",
                                              bufs=2)
                                for h in range(H_LOC):
                                    nc.tensor.matmul(
                                        py[:],
                                        attn_cur[:, h, ntl * P:(ntl + 1) * P],
                                        wo_sb[:, h, oc * CH:(oc + 1) * CH],
                                        start=(h == 0), stop=(h == H_LOC - 1))
                                if i == 0:
                                    nc.scalar.activation(
                                        yt[:, 0:CH], py[:],
                                        mybir.ActivationFunctionType.Copy)
                                else:
                                    nc.vector.tensor_copy(yt[:, CH:2 * CH],
                                                          py[:])
                            nc.sync.dma_start(
                                y3[:, nt, half * D // 2:(half + 1) * D // 2],
                                yt[:])

                # first two score blocks, then v (their exp hides under it)
                scores_block(0)
                scores_block(1)
                for nchunk in range(NCH):
                    nsl = slice(nchunk * CH, (nchunk + 1) * CH)
                    xc = xcp.tile([P, DCH, CH], dt.float16, tag="xc")
                    nc.sync.dma_start(xc[:], xT3[:, :, nsl])
                    for nt in range(NCH):
                        ps = psc.tile([P, CH], dt.float32, tag="pv", bufs=2)
                        for dc in range(DCH):
                            nc.tensor.matmul(
                                ps[:], xc[:, dc, nt * P:(nt + 1) * P],
                                wv_sb[:, dc, :],
                                start=(dc == 0), stop=(dc == DCH - 1))
                        nc.scalar.activation(
                            v_sb[:, nchunk * NCH + nt, :], ps[:],
                            mybir.ActivationFunctionType.Copy)

                NB = NCH * H_LOC
                attn_cur = None
                for b in range(NB):
                    if b % H_LOC == 0:
                        attn_cur = attnp.tile([P, H_LOC, CH], dt.float16,
                                              tag="attn")
                    pv_block(b, attn_cur)
                    if b + 2 < NB:
                        scores_block(b + 2)
                    if b % H_LOC == H_LOC - 1:
                        proj_chunk(b // H_LOC, attn_cur)

    nc.compile()
    return nc


_NC_CACHE = None


def _get_program():
    global _NC_CACHE
    if _NC_CACHE is None:
        _NC_CACHE = _build_program()
    return _NC_CACHE


def _rope_tables():
    scale = np.arange(0, HD, 2, dtype=np.float32) / HD
    inv_freq = 1.0 / (10000.0 ** scale)                 # [64]
    t = np.arange(S, dtype=np.float32)
    ang = np.outer(t, inv_freq)                         # [S, 64]
    cos = np.cos(ang).T.astype(np.float32)              # [64, S]
    sin = np.sin(ang).T.astype(np.float32)
    stk = lambda a: np.ascontiguousarray(
        np.concatenate([a, a], axis=0)).astype(_F16)    # [128, S]
    return stk(cos), stk(sin)


def prepare_in_maps(x, wq, wk, wv, wo):
    x = np.asarray(x, dtype=np.float32)
    wq = np.asarray(wq, dtype=np.float32) * np.float32(1.0 / np.sqrt(HD))
    wk = np.asarray(wk, dtype=np.float32)
    wv = np.asarray(wv, dtype=np.float32)
    wo = np.asarray(wo, dtype=np.float32)

    ct_t, st_t = _rope_tables()

    # even/odd RoPE permutation of rows within each head
    perm = np.concatenate([np.arange(0, HD, 2), np.arange(1, HD, 2)])

    xT = [np.ascontiguousarray(x[b].T).astype(_F16) for b in range(B)]

    in_maps = []
    for c in range(N_CORES):
        b, hg = divmod(c, H_LOC)
        heads = np.arange(hg * H_LOC, (hg + 1) * H_LOC)
        rows_qk = (heads[:, None] * HD + perm[None, :]).reshape(-1)  # [512]
        rows_nat = np.arange(hg * F, (hg + 1) * F)
        in_maps.append({
            "xT": xT[b],
            "wqT": np.ascontiguousarray(wq[rows_qk].T).astype(_F16),
            "wkT": np.ascontiguousarray(wk[rows_qk].T).astype(_F16),
            "wvT": np.ascontiguousarray(wv[rows_nat].T).astype(_F16),
            "woT": np.ascontiguousarray(wo[:, rows_nat].T).astype(_F16),
            "ct": ct_t, "st": st_t,
        })
    return in_maps


def combine_results(results):
    out = np.zeros((B, S, D), dtype=np.float32)
    for c, r in enumerate(results):
        out[c // H_LOC] += r["y"]
    return out


def kernel(x, wq, wk, wv, wo):
    from concourse.bass_utils import run_bass_kernel_spmd

    nc = _get_program()
    in_maps = prepare_in_maps(x, wq, wk, wv, wo)
    res = run_bass_kernel_spmd(nc, in_maps, core_ids=list(range(N_CORES)))
    return combine_results(res.results)


if __name__ == "__main__":
    rng = np.random.default_rng(0)
    ins = {
        "x": rng.standard_normal((B, S, D), dtype=np.float32),
        "wq": rng.standard_normal((D, D), dtype=np.float32) / np.sqrt(D),
        "wk": rng.standard_normal((D, D), dtype=np.float32) / np.sqrt(D),
        "wv": rng.standard_normal((D, D), dtype=np.float32) / np.sqrt(D),
        "wo": rng.standard_normal((D, D), dtype=np.float32) / np.sqrt(D),
    }
    out = kernel(**ins)
    print("out", out.shape, out.dtype, np.abs(out).max())


# revision 11
# speedup vs baseline: 1.2778x; 1.0724x over previous
"""Multi-head attention (B=2, S=2048, D=2048, H=16, hd=128) on 8 TRN2 NeuronCores.

Sharding: data-parallel over batch (2) x tensor-parallel over head groups (4).
Core c handles batch c//4 and heads [4*(c%4), 4*(c%4)+4). Each core computes
q/k/v projections for its 512 features, RoPE, full attention over S for its 4
heads, and a partial output projection y_partial = attn_local @ wo[:, cols].T.
Host sums the 4 partials per batch (no on-chip collectives).

All matmuls run in f16 with fp32 PSUM accumulation. The 1/sqrt(hd) score
scale is folded into wq host-side. RoPE pairs are split even/odd across the
partition dim by permuting wq/wk rows host-side, so RoPE is elementwise DVE
work against stacked [cos;cos] / [sin;sin] tables. Scores are computed
transposed ([k, q]) so softmax(exp)@V needs no on-chip transposes; the
softmax denominator is accumulated on DVE, all-reduced across partitions on
GpSimd, and divided out after the PV matmul.

Emission order is a software pipeline that keeps TensorE dense: k proj,
q proj with the first two attention score blocks interleaved, v proj (exp
hides under the v GEMM), then steady-state
[pv(b) | scores(b+2) | projection(finished q-chunk)].
"""

import numpy as np

B = 2
S = 2048
D = 2048
H = 16
HD = 128
P = 128
N_CORES = 8
H_LOC = 4          # heads per core
F = H_LOC * HD     # local features = 512
NCH = 4            # n-chunks of 512 over S
CH = S // NCH      # 512
DCH = D // P       # 16 contraction chunks
NT = S // P        # 16 row tiles

_F16 = np.float16


def _build_program():
    import concourse.bass_isa as bass_isa
    import concourse.mybir as mybir
    import concourse.tile as tile
    from concourse import bacc

    dt = mybir.dt
    nc = bacc.Bacc("TRN2", target_bir_lowering=False, debug=False,
                   num_devices=N_CORES)

    xT = nc.dram_tensor("xT", [D, S], dt.float16, kind="ExternalInput").ap()
    wqT = nc.dram_tensor("wqT", [D, F], dt.float16, kind="ExternalInput").ap()
    wkT = nc.dram_tensor("wkT", [D, F], dt.float16, kind="ExternalInput").ap()
    wvT = nc.dram_tensor("wvT", [D, F], dt.float16, kind="ExternalInput").ap()
    woT = nc.dram_tensor("woT", [F, D], dt.float16, kind="ExternalInput").ap()
    # stacked RoPE tables: [cos;cos] and [sin;sin]
    ct = nc.dram_tensor("ct", [P, S], dt.float16, kind="ExternalInput").ap()
    st = nc.dram_tensor("st", [P, S], dt.float16, kind="ExternalInput").ap()
    y = nc.dram_tensor("y", [S, D], dt.float32, kind="ExternalOutput").ap()

    xT3 = xT.rearrange("(o p) n -> p o n", p=P)      # [128, 16, 2048]
    wqT3 = wqT.rearrange("(o p) f -> p o f", p=P)    # [128, 16, 512]
    wkT3 = wkT.rearrange("(o p) f -> p o f", p=P)
    wvT3 = wvT.rearrange("(o p) f -> p o f", p=P)
    woT3 = woT.rearrange("(o p) n -> p o n", p=P)    # [128, 4, 2048]
    y3 = y.rearrange("(o p) n -> p o n", p=P)        # [128, 16, 2048]

    NB = NCH * H_LOC  # 16 attention blocks, b = qc*4 + h

    with tile.TileContext(nc) as tc:
        with (
            tc.tile_pool(name="persist", bufs=1) as pp,
            tc.tile_pool(name="xcp", bufs=2) as xcp,
        ):
            qTp = pp.tile([P, H_LOC, S], dt.float16, tag="qTp")
            kTp = pp.tile([P, H_LOC, S], dt.float16, tag="kTp")
            v_sb = pp.tile([P, NT, F], dt.float16, tag="v")
            wv_sb = pp.tile([P, DCH, F], dt.float16, tag="wv")
            wo_sb = pp.tile([P, H_LOC, D], dt.float16, tag="wo")

            # ---- phase 1: k and q projections + RoPE ---------------------
            with (
                tc.tile_pool(name="wp", bufs=1) as wp,
                tc.tile_pool(name="t2p", bufs=3) as t2p,
                tc.tile_pool(name="psg", bufs=1, space="PSUM") as psg,
            ):
                wk_sb = wp.tile([P, DCH, F], dt.float16, tag="wk")
                wq_sb = wp.tile([P, DCH, F], dt.float16, tag="wq")
                ct_sb = wp.tile([P, S], dt.float16, tag="ct")
                st_sb = wp.tile([P, S], dt.float16, tag="st")

                # DMA issue order = need order: wk + first x chunk gate
                # the very first matmul.
                nc.sync.dma_start(wk_sb[:], wkT3[:])
                xc0 = xcp.tile([P, DCH, CH], dt.float16, tag="xc")
                nc.sync.dma_start(xc0[:], xT3[:, :, 0:CH])
                nc.sync.dma_start(ct_sb[:], ct[:])
                nc.sync.dma_start(st_sb[:], st[:])
                nc.sync.dma_start(wq_sb[:], wqT3[:])
                nc.sync.dma_start(wv_sb[:], wvT3[:])
                nc.sync.dma_start(wo_sb[:], woT3[:])

                def proj_rope(w_sb, outT, nchunk, xc=None):
                    """One n-chunk of a q/k projection + RoPE into outT."""
                    nsl = slice(nchunk * CH, (nchunk + 1) * CH)
                    if xc is None:
                        xc = xcp.tile([P, DCH, CH], dt.float16, tag="xc")
                        nc.sync.dma_start(xc[:], xT3[:, :, nsl])
                    for h in range(H_LOC):
                        ps = psg.tile([P, CH], dt.float32, tag="gemm", bufs=3)
                        for dc in range(DCH):
                            nc.tensor.matmul(
                                ps[:], w_sb[:, dc, h * HD:(h + 1) * HD],
                                xc[:, dc, :],
                                start=(dc == 0), stop=(dc == DCH - 1))
                        # RoPE: partitions 0:64 = even pairs e, 64:128 odd o:
                        #   out_e = e*c - o*s ; out_o = e*s + o*c
                        t1 = psg.tile([P, CH], dt.float32, tag="t1", bufs=2)
                        t2 = t2p.tile([P, CH], dt.float16, tag="t2")
                        nc.vector.tensor_mul(out=t1[:], in0=ps[:],
                                             in1=ct_sb[:, nsl])
                        nc.vector.tensor_mul(out=t2[:], in0=ps[:],
                                             in1=st_sb[:, nsl])
                        o_sl = outT[:, h, nsl]
                        nc.vector.tensor_sub(out=o_sl[0:64, :], in0=t1[0:64, :],
                                             in1=t2[64:128, :])
                        nc.vector.tensor_add(out=o_sl[64:128, :],
                                             in0=t2[0:64, :],
                                             in1=t1[64:128, :])

                for nchunk in range(NCH):
                    proj_rope(wk_sb, kTp, nchunk, xc=xc0 if nchunk == 0 else None)
                for nchunk in range(NCH):
                    proj_rope(wq_sb, qTp, nchunk)

            # ---- phase 2: scores pipeline + v + pv + projection ----------
            with (
                tc.tile_pool(name="etp", bufs=16) as etp,
                tc.tile_pool(name="attnp", bufs=2) as attnp,
                tc.tile_pool(name="accp", bufs=3) as accp,
                tc.tile_pool(name="ytp", bufs=4) as ytp,
                tc.tile_pool(name="psc", bufs=1, space="PSUM") as psc,
            ):
                acc_of = {}
                from collections import deque
                sc_iters = deque()

                def scores_gen(b):
                    """Emit one score+exp+acc unit (2 matmuls) per yield, so
                    callers can interleave units with other TensorE work."""
                    qc, h = divmod(b, H_LOC)
                    qsl = slice(qc * CH, (qc + 1) * CH)
                    ets = []
                    acc = accp.tile([P, 2, CH], dt.float16, tag="acc")
                    acc_of[b] = (acc, ets)
                    for ktp in range(NT // 2):
                        ss = psc.tile([P, 2, CH], dt.float32, tag="ss", bufs=2)
                        for i in range(2):
                            kt = 2 * ktp + i
                            nc.tensor.matmul(
                                ss[:, i, :], kTp[:, h, kt * P:(kt + 1) * P],
                                qTp[:, h, qsl], start=True, stop=True)
                        et = etp.tile([P, 2, CH], dt.float16, tag="et")
                        nc.scalar.activation(
                            et[:], ss[:], mybir.ActivationFunctionType.Exp)
                        if ktp == 0:
                            nc.vector.tensor_copy(acc[:], et[:])
                        else:
                            nc.vector.tensor_add(out=acc[:], in0=acc[:],
                                                 in1=et[:])
                        ets.append(et)
                        yield

                def pump(n=1):
                    for _ in range(n):
                        while sc_iters:
                            try:
                                next(sc_iters[0])
                                break
                            except StopIteration:
                                sc_iters.popleft()

                def pv_block(b, attn_cur):
                    qc, h = divmod(b, H_LOC)
                    hsl = slice(h * HD, (h + 1) * HD)
                    acc, ets = acc_of.pop(b)
                    pv = psc.tile([P, CH], dt.float32, tag="pv", bufs=2)
                    for ktp in range(NT // 2):
                        et = ets[ktp]
                        for i in range(2):
                            kt = 2 * ktp + i
                            nc.tensor.matmul(
                                pv[:], v_sb[:, kt, hsl], et[:, i, :],
                                start=(kt == 0), stop=(kt == NT - 1))
                        pump(1)
                    denom = accp.tile([P, CH], dt.float32, tag="den", bufs=2)
                    nc.vector.tensor_add(out=denom[:], in0=acc[:, 0, :],
                                         in1=acc[:, 1, :])
                    allr = accp.tile([P, CH], dt.float32, tag="allr", bufs=2)
                    nc.gpsimd.partition_all_reduce(
                        allr[:], denom[:], channels=P,
                        reduce_op=bass_isa.ReduceOp.add)
                    rec = accp.tile([P, CH], dt.float32, tag="rec", bufs=2)
                    nc.vector.reciprocal_approx_fast(rec[:], allr[:])
                    nc.vector.tensor_mul(
                        out=attn_cur[:, h, :], in0=pv[:], in1=rec[:])

                def proj_chunk(qc, attn_cur):
                    for ntl in range(NCH):
                        nt = qc * NCH + ntl
                        for half in range(2):
                            yt = ytp.tile([P, D // 2], dt.float32, tag="yt")
                            for i in range(2):
                                oc = half * 2 + i
                                py = psc.tile([P, CH], dt.float32, tag="py",
                                              bufs=2)
                                for h in range(H_LOC):
                                    nc.tensor.matmul(
                                        py[:],
                                        attn_cur[:, h, ntl * P:(ntl + 1) * P],
                                        wo_sb[:, h, oc * CH:(oc + 1) * CH],
                                        start=(h == 0), stop=(h == H_LOC - 1))
                                if i == 0:
                                    nc.scalar.activation(
                                        yt[:, 0:CH], py[:],
                                        mybir.ActivationFunctionType.Copy)
                                else:
                                    nc.vector.tensor_copy(yt[:, CH:2 * CH],
                                                          py[:])
                            nc.sync.dma_start(
                                y3[:, nt, half * D // 2:(half + 1) * D // 2],
                                yt[:])

                # v projection, with the first two score blocks pumped
                # in fine-grained units between v PSUM groups
                sc_iters.append(scores_gen(0))
                sc_iters.append(scores_gen(1))
                for nchunk in range(NCH):
                    nsl = slice(nchunk * CH, (nchunk + 1) * CH)
                    xc = xcp.tile([P, DCH, CH], dt.float16, tag="xc")
                    nc.sync.dma_start(xc[:], xT3[:, :, nsl])
                    for nt in range(NCH):
                        ps = psc.tile([P, CH], dt.float32, tag="pv", bufs=2)
                        for dc in range(DCH):
                            nc.tensor.matmul(
                                ps[:], xc[:, dc, nt * P:(nt + 1) * P],
                                wv_sb[:, dc, :],
                                start=(dc == 0), stop=(dc == DCH - 1))
                        nc.scalar.activation(
                            v_sb[:, nchunk * NCH + nt, :], ps[:],
                            mybir.ActivationFunctionType.Copy)
                        pump(1)

                # steady state: [pv(b) | scores(b+2) units | proj(qc-1)]
                NB = NCH * H_LOC
                attn_hist = {}
                for b in range(NB):
                    qc = b // H_LOC
                    if b % H_LOC == 0:
                        attn_hist[qc] = attnp.tile([P, H_LOC, CH], dt.float16,
                                                   tag="attn",
                                                   name=f"attn_{qc}")
                    if b + 2 < NB:
                        sc_iters.append(scores_gen(b + 2))
                    pv_block(b, attn_hist[qc])
                    if b % H_LOC == 0 and b > 0:
                        proj_chunk(qc - 1, attn_hist.pop(qc - 1))
                pump(100)
                proj_chunk(NCH - 1, attn_hist.pop(NCH - 1))

    nc.compile()
    return nc


_NC_CACHE = None


def _get_program():
    global _NC_CACHE
    if _NC_CACHE is None:
        _NC_CACHE = _build_program()
    return _NC_CACHE


def _rope_tables():
    scale = np.arange(0, HD, 2, dtype=np.float32) / HD
    inv_freq = 1.0 / (10000.0 ** scale)                 # [64]
    t = np.arange(S, dtype=np.float32)
    ang = np.outer(t, inv_freq)                         # [S, 64]
    cos = np.cos(ang).T.astype(np.float32)              # [64, S]
    sin = np.sin(ang).T.astype(np.float32)
    stk = lambda a: np.ascontiguousarray(
        np.concatenate([a, a], axis=0)).astype(_F16)    # [128, S]
    return stk(cos), stk(sin)


def prepare_in_maps(x, wq, wk, wv, wo):
    x = np.asarray(x, dtype=np.float32)
    wq = np.asarray(wq, dtype=np.float32) * np.float32(1.0 / np.sqrt(HD))
    wk = np.asarray(wk, dtype=np.float32)
    wv = np.asarray(wv, dtype=np.float32)
    wo = np.asarray(wo, dtype=np.float32)

    ct_t, st_t = _rope_tables()

    # even/odd RoPE permutation of rows within each head
    perm = np.concatenate([np.arange(0, HD, 2), np.arange(1, HD, 2)])

    xT = [np.ascontiguousarray(x[b].T).astype(_F16) for b in range(B)]

    in_maps = []
    for c in range(N_CORES):
        b, hg = divmod(c, H_LOC)
        heads = np.arange(hg * H_LOC, (hg + 1) * H_LOC)
        rows_qk = (heads[:, None] * HD + perm[None, :]).reshape(-1)  # [512]
        rows_nat = np.arange(hg * F, (hg + 1) * F)
        in_maps.append({
            "xT": xT[b],
            "wqT": np.ascontiguousarray(wq[rows_qk].T).astype(_F16),
            "wkT": np.ascontiguousarray(wk[rows_qk].T).astype(_F16),
            "wvT": np.ascontiguousarray(wv[rows_nat].T).astype(_F16),
            "woT": np.ascontiguousarray(wo[:, rows_nat].T).astype(_F16),
            "ct": ct_t, "st": st_t,
        })
    return in_maps


def combine_results(results):
    out = np.zeros((B, S, D), dtype=np.float32)
    for c, r in enumerate(results):
        out[c // H_LOC] += r["y"]
    return out


def kernel(x, wq, wk, wv, wo):
    from concourse.bass_utils import run_bass_kernel_spmd

    nc = _get_program()
    in_maps = prepare_in_maps(x, wq, wk, wv, wo)
    res = run_bass_kernel_spmd(nc, in_maps, core_ids=list(range(N_CORES)))
    return combine_results(res.results)


if __name__ == "__main__":
    rng = np.random.default_rng(0)
    ins = {
        "x": rng.standard_normal((B, S, D), dtype=np.float32),
        "wq": rng.standard_normal((D, D), dtype=np.float32) / np.sqrt(D),
        "wk": rng.standard_normal((D, D), dtype=np.float32) / np.sqrt(D),
        "wv": rng.standard_normal((D, D), dtype=np.float32) / np.sqrt(D),
        "wo": rng.standard_normal((D, D), dtype=np.float32) / np.sqrt(D),
    }
    out = kernel(**ins)
    print("out", out.shape, out.dtype, np.abs(out).max())


# revision 12
# speedup vs baseline: 1.3929x; 1.0900x over previous
"""Multi-head attention (B=2, S=2048, D=2048, H=16, hd=128) on 8 TRN2 NeuronCores.

Sharding: data-parallel over batch (2) x tensor-parallel over head groups (4).
Core c handles batch c//4 and heads [4*(c%4), 4*(c%4)+4). Each core computes
q/k/v projections for its 512 features, RoPE, full attention over S for its 4
heads, and a partial output projection y_partial = attn_local @ wo[:, cols].T.
Host sums the 4 partials per batch (no on-chip collectives).

All matmuls run in f16 with fp32 PSUM accumulation. The 1/sqrt(hd) score
scale is folded into wq host-side. RoPE pairs are split even/odd across the
partition dim by permuting wq/wk rows host-side, so RoPE is elementwise DVE
work against stacked [cos;cos] / [sin;sin] tables. Scores are computed
transposed ([k, q]) so softmax(exp)@V needs no on-chip transposes; the
softmax denominator is accumulated on DVE, all-reduced across partitions on
GpSimd, and divided out after the PV matmul.

Emission order is a software pipeline that keeps TensorE dense: k proj,
q proj with the first two attention score blocks interleaved, v proj (exp
hides under the v GEMM), then steady-state
[pv(b) | scores(b+2) | projection(finished q-chunk)].
"""

import numpy as np

B = 2
S = 2048
D = 2048
H = 16
HD = 128
P = 128
N_CORES = 8
H_LOC = 4          # heads per core
F = H_LOC * HD     # local features = 512
NCH = 4            # n-chunks of 512 over S
CH = S // NCH      # 512
DCH = D // P       # 16 contraction chunks
NT = S // P        # 16 row tiles

_F16 = np.float16


def _build_program():
    import concourse.bass_isa as bass_isa
    import concourse.mybir as mybir
    import concourse.tile as tile
    from concourse import bacc

    dt = mybir.dt
    nc = bacc.Bacc("TRN2", target_bir_lowering=False, debug=False,
                   num_devices=N_CORES)

    xT = nc.dram_tensor("xT", [D, S], dt.float16, kind="ExternalInput").ap()
    wqT = nc.dram_tensor("wqT", [D, F], dt.float16, kind="ExternalInput").ap()
    wkT = nc.dram_tensor("wkT", [D, F], dt.float16, kind="ExternalInput").ap()
    wvT = nc.dram_tensor("wvT", [D, F], dt.float16, kind="ExternalInput").ap()
    woT = nc.dram_tensor("woT", [F, D], dt.float16, kind="ExternalInput").ap()
    # stacked RoPE tables: [cos;cos] and [sin;sin]
    ct = nc.dram_tensor("ct", [P, S], dt.float16, kind="ExternalInput").ap()
    st = nc.dram_tensor("st", [P, S], dt.float16, kind="ExternalInput").ap()
    y = nc.dram_tensor("y", [S, D], dt.float32, kind="ExternalOutput").ap()

    xT3 = xT.rearrange("(o p) n -> p o n", p=P)      # [128, 16, 2048]
    wqT3 = wqT.rearrange("(o p) f -> p o f", p=P)    # [128, 16, 512]
    wkT3 = wkT.rearrange("(o p) f -> p o f", p=P)
    wvT3 = wvT.rearrange("(o p) f -> p o f", p=P)
    woT3 = woT.rearrange("(o p) n -> p o n", p=P)    # [128, 4, 2048]
    y3 = y.rearrange("(o p) n -> p o n", p=P)        # [128, 16, 2048]

    NB = NCH * H_LOC  # 16 attention blocks, b = qc*4 + h

    with tile.TileContext(nc) as tc:
        with (
            tc.tile_pool(name="persist", bufs=1) as pp,
            tc.tile_pool(name="xcp", bufs=3) as xcp,
        ):
            qTp = pp.tile([P, H_LOC, S], dt.float16, tag="qTp")
            kTp = pp.tile([P, H_LOC, S], dt.float16, tag="kTp")
            v_sb = pp.tile([P, NT, F], dt.float16, tag="v")
            wv_sb = pp.tile([P, DCH, F], dt.float16, tag="wv")
            wo_sb = pp.tile([P, H_LOC, D], dt.float16, tag="wo")

            # ---- phase 1: k and q projections + RoPE ---------------------
            with (
                tc.tile_pool(name="wp", bufs=1) as wp,
                tc.tile_pool(name="t2p", bufs=3) as t2p,
                tc.tile_pool(name="psg", bufs=1, space="PSUM") as psg,
            ):
                wk_sb = wp.tile([P, DCH, F], dt.float16, tag="wk")
                wq_sb = wp.tile([P, DCH, F], dt.float16, tag="wq")
                ct_sb = wp.tile([P, S], dt.float16, tag="ct")
                st_sb = wp.tile([P, S], dt.float16, tag="st")

                # DMA issue order = need order: wk + first x chunk gate
                # the very first matmul.
                nc.sync.dma_start(wk_sb[:], wkT3[:])
                xc0 = xcp.tile([P, DCH, CH], dt.float16, tag="xc")
                nc.sync.dma_start(xc0[:], xT3[:, :, 0:CH])
                nc.sync.dma_start(ct_sb[:], ct[:])
                nc.sync.dma_start(st_sb[:], st[:])
                xc1 = xcp.tile([P, DCH, CH], dt.float16, tag="xc")
                nc.sync.dma_start(xc1[:], xT3[:, :, CH:2 * CH])
                nc.sync.dma_start(wq_sb[:], wqT3[:])

                def proj_rope(w_sb, outT, nchunk, xc=None):
                    """One n-chunk of a q/k projection + RoPE into outT."""
                    nsl = slice(nchunk * CH, (nchunk + 1) * CH)
                    if xc is None:
                        xc = xcp.tile([P, DCH, CH], dt.float16, tag="xc")
                        nc.sync.dma_start(xc[:], xT3[:, :, nsl])
                    for h in range(H_LOC):
                        ps = psg.tile([P, CH], dt.float32, tag="gemm", bufs=3)
                        for dc in range(DCH):
                            nc.tensor.matmul(
                                ps[:], w_sb[:, dc, h * HD:(h + 1) * HD],
                                xc[:, dc, :],
                                start=(dc == 0), stop=(dc == DCH - 1))
                        # RoPE: partitions 0:64 = even pairs e, 64:128 odd o:
                        #   out_e = e*c - o*s ; out_o = e*s + o*c
                        t1 = psg.tile([P, CH], dt.float32, tag="t1", bufs=2)
                        t2 = t2p.tile([P, CH], dt.float16, tag="t2")
                        nc.vector.tensor_mul(out=t1[:], in0=ps[:],
                                             in1=ct_sb[:, nsl])
                        nc.vector.tensor_mul(out=t2[:], in0=ps[:],
                                             in1=st_sb[:, nsl])
                        o_sl = outT[:, h, nsl]
                        nc.vector.tensor_sub(out=o_sl[0:64, :], in0=t1[0:64, :],
                                             in1=t2[64:128, :])
                        nc.vector.tensor_add(out=o_sl[64:128, :],
                                             in0=t2[0:64, :],
                                             in1=t1[64:128, :])

                kxc = {0: xc0, 1: xc1}
                for nchunk in range(NCH):
                    proj_rope(wk_sb, kTp, nchunk, xc=kxc.get(nchunk))
                nc.sync.dma_start(wv_sb[:], wvT3[:])
                nc.sync.dma_start(wo_sb[:], woT3[:])
                for nchunk in range(NCH):
                    proj_rope(wq_sb, qTp, nchunk)

            # ---- phase 2: scores pipeline + v + pv + projection ----------
            with (
                tc.tile_pool(name="etp", bufs=16) as etp,
                tc.tile_pool(name="attnp", bufs=2) as attnp,
                tc.tile_pool(name="accp", bufs=3) as accp,
                tc.tile_pool(name="ytp", bufs=4) as ytp,
                tc.tile_pool(name="psc", bufs=1, space="PSUM") as psc,
            ):
                acc_of = {}
                from collections import deque
                sc_iters = deque()

                def scores_gen(b):
                    """Emit one score+exp+acc unit (2 matmuls) per yield, so
                    callers can interleave units with other TensorE work."""
                    qc, h = divmod(b, H_LOC)
                    qsl = slice(qc * CH, (qc + 1) * CH)
                    ets = []
                    acc = accp.tile([P, 2, CH], dt.float16, tag="acc")
                    acc_of[b] = (acc, ets)
                    for ktp in range(NT // 2):
                        ss = psc.tile([P, 2, CH], dt.float32, tag="ss", bufs=2)
                        for i in range(2):
                            kt = 2 * ktp + i
                            nc.tensor.matmul(
                                ss[:, i, :], kTp[:, h, kt * P:(kt + 1) * P],
                                qTp[:, h, qsl], start=True, stop=True)
                        et = etp.tile([P, 2, CH], dt.float16, tag="et")
                        nc.scalar.activation(
                            et[:], ss[:], mybir.ActivationFunctionType.Exp)
                        if ktp == 0:
                            nc.vector.tensor_copy(acc[:], et[:])
                        else:
                            nc.vector.tensor_add(out=acc[:], in0=acc[:],
                                                 in1=et[:])
                        ets.append(et)
                        yield

                def pump(n=1):
                    for _ in range(n):
                        while sc_iters:
                            try:
                                next(sc_iters[0])
                                break
                            except StopIteration:
                                sc_iters.popleft()

                def pv_block(b, attn_cur):
                    qc, h = divmod(b, H_LOC)
                    hsl = slice(h * HD, (h + 1) * HD)
                    acc, ets = acc_of.pop(b)
                    # denominator all-reduce first: acc(b) completed two
                    # blocks ago, so GpSimd overlaps the pv matmuls below and
                    # the reciprocal+scale at the end sees it finished.
                    denom = accp.tile([P, CH], dt.float32, tag="den", bufs=2)
                    nc.vector.tensor_add(out=denom[:], in0=acc[:, 0, :],
                                         in1=acc[:, 1, :])
                    allr = accp.tile([P, CH], dt.float32, tag="allr", bufs=2)
                    nc.gpsimd.partition_all_reduce(
                        allr[:], denom[:], channels=P,
                        reduce_op=bass_isa.ReduceOp.add)
                    pv = psc.tile([P, CH], dt.float32, tag="pv", bufs=2)
                    for ktp in range(NT // 2):
                        et = ets[ktp]
                        for i in range(2):
                            kt = 2 * ktp + i
                            nc.tensor.matmul(
                                pv[:], v_sb[:, kt, hsl], et[:, i, :],
                                start=(kt == 0), stop=(kt == NT - 1))
                        pump(1)
                    rec = accp.tile([P, CH], dt.float32, tag="rec", bufs=2)
                    nc.vector.reciprocal_approx_fast(rec[:], allr[:])
                    nc.vector.tensor_mul(
                        out=attn_cur[:, h, :], in0=pv[:], in1=rec[:])

                def proj_chunk(qc, attn_cur):
                    for ntl in range(NCH):
                        nt = qc * NCH + ntl
                        for half in range(2):
                            yt = ytp.tile([P, D // 2], dt.float32, tag="yt")
                            for i in range(2):
                                oc = half * 2 + i
                                py = psc.tile([P, CH], dt.float32, tag="py",
                                              bufs=2)
                                for h in range(H_LOC):
                                    nc.tensor.matmul(
                                        py[:],
                                        attn_cur[:, h, ntl * P:(ntl + 1) * P],
                                        wo_sb[:, h, oc * CH:(oc + 1) * CH],
                                        start=(h == 0), stop=(h == H_LOC - 1))
                                nc.scalar.activation(
                                    yt[:, i * CH:(i + 1) * CH], py[:],
                                    mybir.ActivationFunctionType.Copy)
                            nc.sync.dma_start(
                                y3[:, nt, half * D // 2:(half + 1) * D // 2],
                                yt[:])

                # v projection, with the first two score blocks pumped
                # in fine-grained units between v PSUM groups
                sc_iters.append(scores_gen(0))
                sc_iters.append(scores_gen(1))
                for nchunk in range(NCH):
                    nsl = slice(nchunk * CH, (nchunk + 1) * CH)
                    xc = xcp.tile([P, DCH, CH], dt.float16, tag="xc")
                    nc.sync.dma_start(xc[:], xT3[:, :, nsl])
                    for nt in range(NCH):
                        ps = psc.tile([P, CH], dt.float32, tag="pv", bufs=2)
                        for dc in range(DCH):
                            nc.tensor.matmul(
                                ps[:], xc[:, dc, nt * P:(nt + 1) * P],
                                wv_sb[:, dc, :],
                                start=(dc == 0), stop=(dc == DCH - 1))
                        nc.scalar.activation(
                            v_sb[:, nchunk * NCH + nt, :], ps[:],
                            mybir.ActivationFunctionType.Copy)
                        pump(1)

                # steady state: [pv(b) | scores(b+2) units | proj(qc-1)]
                NB = NCH * H_LOC
                attn_hist = {}
                for b in range(NB):
                    qc = b // H_LOC
                    if b % H_LOC == 0:
                        attn_hist[qc] = attnp.tile([P, H_LOC, CH], dt.float16,
                                                   tag="attn",
                                                   name=f"attn_{qc}")
                    if b + 2 < NB:
                        sc_iters.append(scores_gen(b + 2))
                    pv_block(b, attn_hist[qc])
                    if b % H_LOC == 0 and b > 0:
                        proj_chunk(qc - 1, attn_hist.pop(qc - 1))
                pump(100)
                proj_chunk(NCH - 1, attn_hist.pop(NCH - 1))

    nc.compile()
    return nc


_NC_CACHE = None


def _get_program():
    global _NC_CACHE
    if _NC_CACHE is None:
        _NC_CACHE = _build_program()
    return _NC_CACHE


def _rope_tables():
    scale = np.arange(0, HD, 2, dtype=np.float32) / HD
    inv_freq = 1.0 / (10000.0 ** scale)                 # [64]
    t = np.arange(S, dtype=np.float32)
    ang = np.outer(t, inv_freq)                         # [S, 64]
    cos = np.cos(ang).T.astype(np.float32)              # [64, S]
    sin = np.sin(ang).T.astype(np.float32)
    stk = lambda a: np.ascontiguousarray(
        np.concatenate([a, a], axis=0)).astype(_F16)    # [128, S]
    return stk(cos), stk(sin)


def prepare_in_maps(x, wq, wk, wv, wo):
    x = np.asarray(x, dtype=np.float32)
    wq = np.asarray(wq, dtype=np.float32) * np.float32(1.0 / np.sqrt(HD))
    wk = np.asarray(wk, dtype=np.float32)
    wv = np.asarray(wv, dtype=np.float32)
    wo = np.asarray(wo, dtype=np.float32)

    ct_t, st_t = _rope_tables()

    # even/odd RoPE permutation of rows within each head
    perm = np.concatenate([np.arange(0, HD, 2), np.arange(1, HD, 2)])

    xT = [np.ascontiguousarray(x[b].T).astype(_F16) for b in range(B)]

    in_maps = []
    for c in range(N_CORES):
        b, hg = divmod(c, H_LOC)
        heads = np.arange(hg * H_LOC, (hg + 1) * H_LOC)
        rows_qk = (heads[:, None] * HD + perm[None, :]).reshape(-1)  # [512]
        rows_nat = np.arange(hg * F, (hg + 1) * F)
        in_maps.append({
            "xT": xT[b],
            "wqT": np.ascontiguousarray(wq[rows_qk].T).astype(_F16),
            "wkT": np.ascontiguousarray(wk[rows_qk].T).astype(_F16),
            "wvT": np.ascontiguousarray(wv[rows_nat].T).astype(_F16),
            "woT": np.ascontiguousarray(wo[:, rows_nat].T).astype(_F16),
            "ct": ct_t, "st": st_t,
        })
    return in_maps


def combine_results(results):
    out = np.zeros((B, S, D), dtype=np.float32)
    for c, r in enumerate(results):
        out[c // H_LOC] += r["y"]
    return out


def kernel(x, wq, wk, wv, wo):
    from concourse.bass_utils import run_bass_kernel_spmd

    nc = _get_program()
    in_maps = prepare_in_maps(x, wq, wk, wv, wo)
    res = run_bass_kernel_spmd(nc, in_maps, core_ids=list(range(N_CORES)))
    return combine_results(res.results)


if __name__ == "__main__":
    rng = np.random.default_rng(0)
    ins = {
        "x": rng.standard_normal((B, S, D), dtype=np.float32),
        "wq": rng.standard_normal((D, D), dtype=np.float32) / np.sqrt(D),
        "wk": rng.standard_normal((D, D), dtype=np.float32) / np.sqrt(D),
        "wv": rng.standard_normal((D, D), dtype=np.float32) / np.sqrt(D),
        "wo": rng.standard_normal((D, D), dtype=np.float32) / np.sqrt(D),
    }
    out = kernel(**ins)
    print("out", out.shape, out.dtype, np.abs(out).max())
